# revision 1
# baseline (speedup 1.0000x reference)
"""CosArcLoss on 8 TRN2 NeuronCores (Bass/Tile).

Math (reference, f32):
    t_i   = preds[i, labels[i]]
    theta = arccos(clip(t_i, -1+1e-12, 1-1e-12))    # == clip(t_i,-1,1) in f32
    num_i = 30*(cos(theta + 0.5) - 0.35)
          = 30*cos(0.5)*t_i - 30*sin(0.5)*sqrt(1-t_i^2) - 10.5
    S_i   = sum_j exp(30*preds[i,j])
    den_i = exp(num_i) + S_i - exp(30*t_i)
    loss  = mean_i( log(den_i) - num_i )

Sharding: batch-parallel, 256 rows/core. Each row is rotated on the host so
its target column sits at local column 0 (row sums are rotation-invariant),
making the device program a pure streaming exp+rowsum with a tiny epilogue
and no gather / no collective. Final mean over the 8*[128,2] per-row losses
happens on the host (the "all-reduce" of the unshard step).

Schedule notes: the numerator chain (sqrt etc.) depends only on the target
column, so it is emitted first — its ACT table loads hide under the first
x-tile DMA. Deep x-tile buffering (bufs=8) keeps the DMA queue full so the
streaming phase is HBM-bound; ScalarE does exp + row-sum (accum_out) at
~1 elem/cycle/lane, below the DMA rate.
"""
import numpy as np
from contextlib import ExitStack

import concourse.bass as bass
import concourse.tile as tile
from concourse import bacc, mybir
from concourse.bass_utils import run_bass_kernel_spmd

B, V = 2048, 32000
N_CORES = 8
RPC = B // N_CORES            # 256 rows per core
P = 128                       # SBUF partitions
G = RPC // P                  # 2 row groups per core

# column tiling: small leading tiles (fast ScalarE start) for group 0,
# reversed for group 1 so the stream also ENDS on small tiles (short tail)
TILES = [500, 1500, 2000] + [4000] * 7
assert sum(TILES) == V
NT = len(TILES)
GTILES = [list(TILES), list(reversed(TILES))]

SCALE = 30.0
CM = SCALE * np.cos(0.5)      # 26.327476856711183
SM = SCALE * np.sin(0.5)      # 14.38276615812609
CB = SCALE * 0.35             # 10.5

F32 = mybir.dt.float32
AF = mybir.ActivationFunctionType
ALU = mybir.AluOpType

_cache = {}


def _build():
    nc = bacc.Bacc("TRN2", target_bir_lowering=False, debug=False,
                   num_devices=N_CORES)
    x = nc.dram_tensor("x", [RPC, V], F32, kind="ExternalInput")
    # out[:, 0:G] = den, out[:, G:2G] = num; the final ln(den)-num over the
    # 2048 per-row pairs happens host-side (saves the tail's ln-table load)
    out = nc.dram_tensor("out", [P, 2 * G], F32, kind="ExternalOutput")

    with tile.TileContext(nc) as tc, ExitStack() as ctx:
        xpool = ctx.enter_context(tc.tile_pool(name="x", bufs=8))
        epool = ctx.enter_context(tc.tile_pool(name="e", bufs=2))
        spool = ctx.enter_context(tc.tile_pool(name="s", bufs=1))

        ssum = spool.tile([P, G * NT], F32)   # per-(group,tile) exp row-sums
        tvec = spool.tile([P, G], F32)        # target logits t

        # --- target column + sqrt chain, emitted pre-stream: its ACT table
        # loads land in the ramp shadow while the first x tiles stream in ---
        with tc.high_priority():
            for g in range(G):
                nc.sync.dma_start(tvec[:, g:g + 1], x[g * P:(g + 1) * P, 0:1])

            tsq = spool.tile([P, G], F32)
            nc.vector.tensor_mul(tsq[:], tvec[:], tvec[:])
            omts = spool.tile([P, G], F32)
            # (t^2 * -1) + 1, clamped away from 0 for the sqrt
            nc.vector.tensor_scalar(omts[:], tsq[:], -1.0, 1.0,
                                    ALU.mult, ALU.add)
            omc = spool.tile([P, G], F32)
            nc.vector.tensor_scalar_max(omc[:], omts[:], 1e-30)
            r = spool.tile([P, G], F32)
            nc.scalar.activation(r[:], omc[:], AF.Sqrt)

        # --- streaming pass: exp(30 x) + per-row sums on ScalarE ---
        for g in range(G):
            rs = slice(g * P, (g + 1) * P)
            off = 0
            for t, tc_ in enumerate(GTILES[g]):
                xt = xpool.tile([P, tc_], F32, tag="xt")
                nc.sync.dma_start(xt[:], x[rs, off:off + tc_])
                et = epool.tile([P, tc_], F32, tag="et")
                nc.scalar.activation(
                    et[:], xt[:], AF.Exp, scale=SCALE,
                    accum_out=ssum[:, g * NT + t: g * NT + t + 1],
                )
                off += tc_

        # --- numerator epilogue (gap-fills into the stream; exp set stays) ---
        a = spool.tile([P, G], F32)
        nc.vector.tensor_scalar(a[:], tvec[:], float(CM), -float(CB),
                                ALU.mult, ALU.add)
        bb = spool.tile([P, G], F32)
        nc.vector.tensor_scalar_mul(bb[:], r[:], float(SM))
        num = spool.tile([P, G], F32)
        nc.vector.tensor_sub(num[:], a[:], bb[:])

        enum_ = spool.tile([P, G], F32)
        nc.scalar.activation(enum_[:], num[:], AF.Exp)
        e30t = spool.tile([P, G], F32)
        nc.scalar.activation(e30t[:], tvec[:], AF.Exp, scale=SCALE)
        # exp(num) - exp(30 t), folded before S arrives
        ed = spool.tile([P, G], F32)
        nc.vector.tensor_sub(ed[:], enum_[:], e30t[:])

        # --- tail: S, den, loss ---
        S = spool.tile([P, G], F32)
        for g in range(G):
            nc.vector.tensor_reduce(
                S[:, g:g + 1], ssum[:, g * NT:(g + 1) * NT],
                axis=mybir.AxisListType.X, op=ALU.add,
            )
        dn = spool.tile([P, 2 * G], F32)
        nc.vector.tensor_add(dn[:, 0:G], S[:], ed[:])
        nc.vector.tensor_copy(dn[:, G:2 * G], num[:])

        nc.sync.dma_start(out[:, :], dn[:])

    nc.compile()
    return nc


def _get_nc():
    if "nc" not in _cache:
        _cache["nc"] = _build()
    return _cache["nc"]


def _shard(preds, labels):
    """Rotate each row so its target column lands at column 0; split by core."""
    preds = np.ascontiguousarray(preds, dtype=np.float32)
    labels = np.asarray(labels).astype(np.int64)
    in_maps = []
    for c in range(N_CORES):
        shard = np.empty((RPC, V), np.float32)
        for i in range(RPC):
            r = c * RPC + i
            l = int(labels[r])
            shard[i, :V - l] = preds[r, l:]
            shard[i, V - l:] = preds[r, :l]
        in_maps.append({"x": shard})
    return in_maps


def kernel(preds, labels):
    in_maps = _shard(preds, labels)
    nc = _get_nc()
    res = run_bass_kernel_spmd(nc, in_maps, list(range(N_CORES)))
    total = 0.0
    for c in range(N_CORES):
        o = np.asarray(res.results[c]["out"], np.float64)
        den, num = o[:, :G], o[:, G:]
        total += (np.log(den) - num).sum()
    return np.array(total / B, dtype=np.float32)



# revision 2
# speedup vs baseline: 1.1436x; 1.1436x over previous
"""CosArcLoss on 8 TRN2 NeuronCores (Bass/Tile).

Math (reference, f32):
    t_i   = preds[i, labels[i]]
    theta = arccos(clip(t_i, -1+1e-12, 1-1e-12))
    num_i = 30*(cos(theta + 0.5) - 0.35)
    S_i   = sum_j exp(30*preds[i,j])
    den_i = exp(num_i) + S_i - exp(30*t_i)
    loss  = mean_i( log(den_i) - num_i )

Everything except S_i is O(B) work and happens on the host in f64 during the
unshard step. The device program is a pure streaming exp(30x) + row-sum:
batch-parallel, 256 rows/core, viewed as [128 partitions, 64000] so each
partition streams two full rows back-to-back (zero-copy shard: a reshape of
a contiguous row-slice of preds). No gather, no collective, no rotation.

Schedule: one flat DMA stream of 12 column tiles per partition. Big tiles
(8000 cols = 32 KiB packets per partition line) amortize per-packet DMA
overhead; the tail tiles shrink (4000..500) so the final exp after the last
byte lands is short. ScalarE does exp + per-tile row accumulation
(accum_out); its throughput (~0.8 elem/cyc/lane) is ~20% above the DMA
delivery rate, so the stream is DMA-bound end-to-end. Tile boundaries never
straddle the mid-partition row boundary (col 32000), so each per-tile accum
belongs to exactly one of the partition's two rows; the host sums tiles
0..3 -> local row 2p, tiles 4..11 -> local row 2p+1, in f64.
"""
import numpy as np
from contextlib import ExitStack

import concourse.bass as bass
import concourse.tile as tile
from concourse import bacc, mybir
from concourse.bass_utils import run_bass_kernel_spmd

B, V = 2048, 32000
N_CORES = 8
RPC = B // N_CORES            # 256 rows per core
P = 128                       # SBUF partitions
W = RPC * V // P              # 64000 cols per partition (= 2 rows)

# column tiling of the per-partition stream; boundaries avoid col 32000
TILES = [8000, 8000, 8000, 8000,                     # local row 2p
         8000, 8000, 8000, 4000, 2000, 1000, 500, 500]  # local row 2p+1
assert sum(TILES) == W
NT = len(TILES)
NT_A = 4                      # tiles 0..NT_A-1 lie in the first row

SCALE = 30.0

F32 = mybir.dt.float32
AF = mybir.ActivationFunctionType

_cache = {}


def _build():
    nc = bacc.Bacc("TRN2", target_bir_lowering=False, debug=False,
                   num_devices=N_CORES)
    x = nc.dram_tensor("x", [P, W], F32, kind="ExternalInput")
    out = nc.dram_tensor("out", [P, NT], F32, kind="ExternalOutput")

    with tile.TileContext(nc) as tc, ExitStack() as ctx:
        xpool = ctx.enter_context(tc.tile_pool(name="x", bufs=4))
        epool = ctx.enter_context(tc.tile_pool(name="e", bufs=1))
        spool = ctx.enter_context(tc.tile_pool(name="s", bufs=1))

        ssum = spool.tile([P, NT], F32)   # per-tile exp row-sums

        off = 0
        for t, tc_ in enumerate(TILES):
            xt = xpool.tile([P, tc_], F32, tag="xt")
            nc.sync.dma_start(xt[:], x[:, off:off + tc_])
            et = epool.tile([P, tc_], F32, tag="et")
            nc.scalar.activation(
                et[:], xt[:], AF.Exp, scale=SCALE,
                accum_out=ssum[:, t:t + 1],
            )
            off += tc_

        nc.sync.dma_start(out[:, :], ssum[:])

    nc.compile()
    return nc


def _get_nc():
    if "nc" not in _cache:
        _cache["nc"] = _build()
    return _cache["nc"]


def _shard(preds, labels=None):
    """Zero-copy: core c gets rows [c*256, (c+1)*256) viewed as [128, 64000]."""
    preds = np.ascontiguousarray(preds, dtype=np.float32)
    return [{"x": preds[c * RPC:(c + 1) * RPC].reshape(P, W)}
            for c in range(N_CORES)]


def kernel(preds, labels):
    preds = np.ascontiguousarray(np.asarray(preds), dtype=np.float32)
    labels = np.asarray(labels).astype(np.int64)
    nc = _get_nc()
    res = run_bass_kernel_spmd(nc, _shard(preds), list(range(N_CORES)))

    # unshard: per-row exp-sums S_i, combined in f64
    S = np.empty(B, np.float64)
    p = np.arange(P)
    for c in range(N_CORES):
        o = np.asarray(res.results[c]["out"], np.float64)   # [P, NT]
        S[c * RPC + 2 * p] = o[:, :NT_A].sum(axis=1)        # local rows 2p
        S[c * RPC + 2 * p + 1] = o[:, NT_A:].sum(axis=1)    # local rows 2p+1

    # host epilogue (f64, O(B)): numerator + target correction + mean
    t = preds[np.arange(B), labels].astype(np.float64)
    eps = 1e-12
    theta = np.arccos(np.clip(t, -1.0 + eps, 1.0 - eps))
    theta = np.clip(theta, eps, np.pi - eps)
    num = SCALE * (np.cos(theta + 0.5) - 0.35)
    den = np.exp(num) + S - np.exp(SCALE * t)
    loss = -(num - np.log(den)).mean()
    return np.array(loss, dtype=np.float32)


# revision 4
# speedup vs baseline: 1.1710x; 1.0239x over previous
"""CosArcLoss on 8 TRN2 NeuronCores (Bass/Tile).

Math (reference, f32):
    t_i   = preds[i, labels[i]]
    theta = arccos(clip(t_i, -1+1e-12, 1-1e-12))
    num_i = 30*(cos(theta + 0.5) - 0.35)
    S_i   = sum_j exp(30*preds[i,j])
    den_i = exp(num_i) + S_i - exp(30*t_i)
    loss  = mean_i( log(den_i) - num_i )

Everything except S_i is O(B) work and happens on the host in f64 during the
unshard step. The device program is a pure streaming exp(30x) + row-sum:
batch-parallel, 256 rows/core, viewed as [128 partitions, 64000] so each
partition streams two full rows back-to-back (zero-copy shard: a reshape of
a contiguous row-slice of preds). No gather, no collective, no rotation.

Schedule: one flat DMA stream of 12 column tiles per partition. Big tiles
(8000 cols = 32 KiB packets per partition line) amortize per-packet DMA
overhead; the tail tiles shrink (4000..500) so the final exp after the last
byte lands is short. ScalarE does exp + per-tile row accumulation
(accum_out); its throughput (~0.8 elem/cyc/lane) is ~20% above the DMA
delivery rate, so the stream is DMA-bound end-to-end. Tile boundaries never
straddle the mid-partition row boundary (col 32000), so each per-tile accum
belongs to exactly one of the partition's two rows; the host sums tiles
0..3 -> local row 2p, tiles 4..11 -> local row 2p+1, in f64.
"""
import numpy as np
from contextlib import ExitStack

import concourse.bass as bass
import concourse.tile as tile
from concourse import bacc, mybir
from concourse.bass_utils import run_bass_kernel_spmd

B, V = 2048, 32000
N_CORES = 8
RPC = B // N_CORES            # 256 rows per core
P = 128                       # SBUF partitions
W = RPC * V // P              # 64000 cols per partition (= 2 rows)

# column tiling of the per-partition stream; boundaries avoid col 32000.
# small leading tiles let ScalarE start ~9us into the stream instead of
# waiting for a huge first tile; the shrinking tail keeps the final exp
# after the last byte short. 6000 cols = 24KB packets (full DMA rate).
TILES = [1500, 2500, 5000, 5000, 6000, 6000, 6000,           # local row 2p
         6000, 6000, 6000, 6000, 4500, 2000, 1000, 500]      # local row 2p+1
assert sum(TILES[:7]) == V and sum(TILES[7:]) == V
NT = len(TILES)
NT_A = 7                      # tiles 0..NT_A-1 lie in the first row

SCALE = 30.0

F32 = mybir.dt.float32
AF = mybir.ActivationFunctionType

_cache = {}


def _build():
    nc = bacc.Bacc("TRN2", target_bir_lowering=False, debug=False,
                   num_devices=N_CORES)
    x = nc.dram_tensor("x", [P, W], F32, kind="ExternalInput")
    out = nc.dram_tensor("out", [P, NT], F32, kind="ExternalOutput")

    with tile.TileContext(nc) as tc, ExitStack() as ctx:
        xpool = ctx.enter_context(tc.tile_pool(name="x", bufs=6))
        epool = ctx.enter_context(tc.tile_pool(name="e", bufs=1))
        spool = ctx.enter_context(tc.tile_pool(name="s", bufs=1))

        ssum = spool.tile([P, NT], F32)   # per-tile exp row-sums

        off = 0
        for t, tc_ in enumerate(TILES):
            xt = xpool.tile([P, tc_], F32, tag="xt")
            nc.sync.dma_start(xt[:], x[:, off:off + tc_])
            et = epool.tile([P, tc_], F32, tag="et")
            nc.scalar.activation(
                et[:], xt[:], AF.Exp, scale=SCALE,
                accum_out=ssum[:, t:t + 1],
            )
            off += tc_

        nc.sync.dma_start(out[:, :], ssum[:])

    nc.compile()
    return nc


def _get_nc():
    if "nc" not in _cache:
        _cache["nc"] = _build()
    return _cache["nc"]


def _shard(preds, labels=None):
    """Zero-copy: core c gets rows [c*256, (c+1)*256) viewed as [128, 64000]."""
    preds = np.ascontiguousarray(preds, dtype=np.float32)
    return [{"x": preds[c * RPC:(c + 1) * RPC].reshape(P, W)}
            for c in range(N_CORES)]


def kernel(preds, labels):
    preds = np.ascontiguousarray(np.asarray(preds), dtype=np.float32)
    labels = np.asarray(labels).astype(np.int64)
    nc = _get_nc()
    res = run_bass_kernel_spmd(nc, _shard(preds), list(range(N_CORES)))

    # unshard: per-row exp-sums S_i, combined in f64
    S = np.empty(B, np.float64)
    p = np.arange(P)
    for c in range(N_CORES):
        o = np.asarray(res.results[c]["out"], np.float64)   # [P, NT]
        S[c * RPC + 2 * p] = o[:, :NT_A].sum(axis=1)        # local rows 2p
        S[c * RPC + 2 * p + 1] = o[:, NT_A:].sum(axis=1)    # local rows 2p+1

    # host epilogue (f64, O(B)): numerator + target correction + mean
    t = preds[np.arange(B), labels].astype(np.float64)
    eps = 1e-12
    theta = np.arccos(np.clip(t, -1.0 + eps, 1.0 - eps))
    theta = np.clip(theta, eps, np.pi - eps)
    num = SCALE * (np.cos(theta + 0.5) - 0.35)
    den = np.exp(num) + S - np.exp(SCALE * t)
    loss = -(num - np.log(den)).mean()
    return np.array(loss, dtype=np.float32)
